# revision 100
# baseline (speedup 1.0000x reference)
"""RBF-kernel causal attention on 8 Trainium2 NeuronCores.

B=2, H=16, N=2048, D=64. Shards the 32 (b,h) attention instances across 8
cores (4 heads per core). Math notes:

  logits = -relu(||q-k||^2)/sqrt(D); relu is a no-op (||q-k||^2 >= 0 up to
  rounding), and softmax is invariant to per-query offsets, so
      softmax_n(-(qsq_m + ksq_n - 2 qk)/8) == softmax_n(qk/4 - ksq_n/8)
  The per-key term is folded into the QK matmul itself: k tiles are extended
  with a 65th row holding ksq_n and q tiles with a 65th row holding -0.5, so
  the 65-partition contraction directly yields qk - ksq/2, and
      P = exp(0.25 * (K Q^T - ksq/2))        in a [key, query] layout.
  V is extended with a ones column (vaug, fp16); the PV step runs P tiles as
  the STATIONARY operand ([128 key, 128 query] fp16) against vaug as the
  moving operand ([128 key, 65]), accumulating O[query, d] | l[query] directly
  in natural orientation -- no output transpose, and only 65 moving rows per
  key tile.  Final output O[m,d] = acc[m,d] / l[m].

Emission is manually software-pipelined: head h+1's setup chunks (transposes,
ksq, vaug build) are interleaved between head h's query blocks so the tile
scheduler (limited lookahead) can overlap them.
"""

import sys

if "/opt/trn_rl_repo" not in sys.path:
    sys.path.insert(0, "/opt/trn_rl_repo")

import numpy as np

import concourse.bacc as bacc
import concourse.mybir as mybir
import concourse.tile as tile
B, H, N, D = 2, 16, 2048, 64
NCORES = 8
HPC = (B * H) // NCORES  # heads per core = 4
P = 128                  # partitions
NT = N // P              # key tiles per head = 16
QB = 512                 # query block = 4 query sub-tiles of 128
MBS = N // QB            # query blocks per head = 4
G = 2                    # key tiles per exp/ACT group (2 PSUM banks)
DE = D + 1               # extended depth (65): ksq row / ones column

F32 = mybir.dt.float32
# float32r = relaxed-precision fp32 matmul (1 cycle/row at moving dim >= 256
# instead of 4 for float32); bit-identical data to f32.
MM_DT = mybir.dt.float32r
HALF = mybir.dt.float16  # fp16: same matmul/DVE speed as bf16, 8x the mantissa


def build_nc():
    nc = bacc.Bacc("TRN2", target_bir_lowering=False, debug=False)
    q = nc.dram_tensor("q", [HPC, N, D], F32, kind="ExternalInput")
    k = nc.dram_tensor("k", [HPC, N, D], F32, kind="ExternalInput")
    v = nc.dram_tensor("v", [HPC, N, D], F32, kind="ExternalInput")
    out = nc.dram_tensor("out", [HPC, N, D], F32, kind="ExternalOutput")

    with tile.TileContext(nc) as tc:
        with (
            tc.tile_pool(name="const", bufs=1) as const_pool,
            tc.tile_pool(name="loads", bufs=1) as load_pool,
            tc.tile_pool(name="head", bufs=3) as head_pool,
            tc.tile_pool(name="work", bufs=4) as work_pool,
            tc.tile_pool(name="p", bufs=4) as p_pool,
            tc.tile_pool(name="epi", bufs=6) as epi_pool,
            tc.tile_pool(name="st", bufs=2, space="PSUM") as st_pool,
            tc.tile_pool(name="tpp", bufs=2, space="PSUM") as tp_pool,
            tc.tile_pool(name="otp", bufs=2, space="PSUM") as ot_pool,
        ):
            # identity on DVE (Pool is busy with other startup memsets) and a
            # warm-up transpose right behind it: the PE clock ramps to full
            # speed only after ~3us of busy history, so starting the ramp at
            # ~1us makes the real transposes and first QKs run 2x faster
            identity = const_pool.tile([P, P], F32)
            nc.vector.memset(identity[:], 0.0)
            nc.gpsimd.affine_select(
                out=identity[:], in_=identity[:],
                compare_op=mybir.AluOpType.not_equal, fill=1.0,
                base=0, pattern=[[-1, P]], channel_multiplier=1,
            )
            wtp = tp_pool.tile([DE, 4, P], F32, tag="tp", name="wtp")
            nc.tensor.transpose(wtp[:, 0, :], identity[:, :DE], identity[:])
            # triangular causal mask for the diagonal squares: the PV step
            # only reads pgd[:, jj, j*128:(j+1)*128] for j >= jj, and only the
            # j == jj square intersects the causal boundary -- so one shared
            # [128, 128] mask (keep iff m - n >= 0) covers every diag tile.
            tri = const_pool.tile([P, P], HALF, tag="tri", name="tri")
            nc.gpsimd.memset(tri[:], 1.0)
            nc.gpsimd.affine_select(
                out=tri[:], in_=tri[:],
                compare_op=mybir.AluOpType.is_ge, fill=0.0,
                base=0, pattern=[[1, P]], channel_multiplier=-1,
            )

            # prefetch every head's inputs up front: no-wait DMAs stream in
            # the background while compute proceeds.  k/q land in the low 64
            # columns of 65-wide extended tiles (col 64 is filled on-chip).
            kexts, qexts, vtmps = [], [], []
            for h in range(HPC):
                kext = load_pool.tile([P, NT, DE], F32, tag=f"kext{h}")
                qext = load_pool.tile([P, NT, DE], F32, tag=f"qext{h}")
                vtmp = load_pool.tile([P, NT, D], F32, tag=f"vtmp{h}")
                kq = k[h].rearrange("(t p) d -> p t d", p=P)
                qq = q[h].rearrange("(t p) d -> p t d", p=P)
                vq = v[h].rearrange("(t p) d -> p t d", p=P)
                if h == 0:
                    # quarter-granular, k/q prioritized so the first
                    # transposes can start after ~2 DMAs, v one quarter behind
                    order = [("k", 0, 2), ("q", 0, 2), ("k", 2, 2),
                             ("q", 2, 2), ("k", 4, 4), ("q", 4, 4),
                             ("v", 0, 4), ("k", 8, 4), ("q", 8, 4),
                             ("v", 4, 4), ("k", 12, 4), ("q", 12, 4),
                             ("v", 8, 4), ("v", 12, 4)]
                    for which, t0, nt in order:
                        ts = slice(t0, t0 + nt)
                        if which == "k":
                            nc.sync.dma_start(kext[:, ts, :D], kq[:, ts, :])
                        elif which == "q":
                            nc.sync.dma_start(qext[:, ts, :D], qq[:, ts, :])
                        else:
                            nc.sync.dma_start(vtmp[:, ts, :], vq[:, ts, :])
                # heads >= 1 issue their DMAs lazily from prep_chunk: a
                # whole-head DMA would monopolize the DMA engines for ~3us
                # and starve the per-job output stores queued behind it
                kexts.append(kext)
                qexts.append(qext)
                vtmps.append(vtmp)

            heads = [{} for _ in range(HPC)]

            def setup_chunks(h):
                """Emission chunks for head h's setup, in dependency order."""
                st = heads[h]
                kext, qext, vtmp = kexts[h], qexts[h], vtmps[h]

                def allocs():
                    # kt/qt: transposed 65-row operands [d(+ksq | -0.5), keys]
                    st["kt"] = head_pool.tile([DE, NT, P], MM_DT, tag="kt", name="kt")
                    st["qt"] = head_pool.tile([DE, NT, P], MM_DT, tag="qt", name="qt")
                    st["vaug"] = head_pool.tile(
                        [P, NT, DE], HALF, tag="vaug", name="vaug"
                    )
                    # constant 65th rows: q gets -0.5 (so ksq*q65 = -ksq/2),
                    # vaug gets the ones column for the softmax denominator
                    nc.gpsimd.memset(qext[:, :, D], -0.5)
                    nc.gpsimd.memset(st["vaug"][:, :, D], 1.0)

                def prep_chunk(t0, nt=4):
                    # ksq + vaug build for tiles [t0, t0+nt), square on Pool
                    # (DVE stays free for the per-job mask/copy work): runs as
                    # soon as those tiles of k and v have landed.  Head 0
                    # squares on DVE instead -- it gates the whole startup and
                    # Pool is busy building the identity/tri consts then.
                    sq_eng = nc.vector if h == 0 else nc.gpsimd

                    def run():
                        ts = slice(t0, t0 + nt)
                        if h > 0:
                            # lazy quarter loads (see prefetch note above)
                            kq = k[h].rearrange("(t p) d -> p t d", p=P)
                            qq = q[h].rearrange("(t p) d -> p t d", p=P)
                            vq = v[h].rearrange("(t p) d -> p t d", p=P)
                            nc.sync.dma_start(kext[:, ts, :D], kq[:, ts, :])
                            nc.sync.dma_start(qext[:, ts, :D], qq[:, ts, :])
                            nc.sync.dma_start(vtmp[:, ts, :], vq[:, ts, :])
                        ktmp = work_pool.tile([P, 4, D], F32, tag="ktmp")
                        sq_eng.tensor_mul(
                            out=ktmp[:, :nt], in0=kext[:, ts, :D], in1=kext[:, ts, :D]
                        )
                        nc.vector.tensor_reduce(
                            kext[:, ts, D], ktmp[:, :nt],
                            axis=mybir.AxisListType.X, op=mybir.AluOpType.add,
                        )
                        nc.gpsimd.tensor_copy(
                            out=st["vaug"][:, ts, :D], in_=vtmp[:, ts, :]
                        )

                    return run

                def tr_group(which, t0, nt=4, act_copy=False):
                    # nt transposes: [128 n, 65] -> [65, 128 n].  act_copy
                    # routes the PSUM->SBUF copy through the (startup-idle)
                    # ACT engine to keep DVE free for the ksq chain.
                    def run():
                        src = kext if which == "k" else qext
                        dst = st["kt" if which == "k" else "qt"]
                        tp = tp_pool.tile([DE, 4, P], F32, tag="tp", name="tp")
                        for j in range(nt):
                            nc.tensor.transpose(
                                tp[:, j, :], src[:, t0 + j, :], identity[:]
                            )
                        if act_copy:
                            nc.scalar.copy(
                                out=dst[:, t0 : t0 + nt, :], in_=tp[:, :nt]
                            )
                        else:
                            nc.vector.tensor_copy(
                                out=dst[:, t0 : t0 + nt, :], in_=tp[:, :nt]
                            )

                    return run

                def ktr_group(g):
                    return tr_group("k", 4 * g)

                def qtr_group(g):
                    return tr_group("q", 4 * g)

                def tr_pairs(c):
                    # 2-tile transpose chunks: shorter PE bursts interleave
                    # with job QKs without starving the exp pipeline
                    for t0 in (4 * c, 4 * c + 2):
                        yield tr_group("q", t0, 2)
                        yield tr_group("k", t0, 2)

                # query block mb needs kt tiles <= 4mb+3, qt group mb, vaug
                # quarter mb; yield in dependency order.  ksq (prep) gates
                # the k transposes, so preps lead their quarter's groups.
                yield allocs
                if h == 0:
                    # startup critical path: 2-tile bites; the ksq chain
                    # (DMA -> square -> reduce -> transpose -> copy) is the
                    # long pole, so it leads and stays uninterrupted on DVE
                    # while the q copies ride the idle ACT engine
                    yield prep_chunk(0, 2)
                    yield tr_group("k", 0, 2)
                    yield tr_group("q", 0, 2, act_copy=True)
                    yield prep_chunk(2, 2)
                    yield tr_group("k", 2, 2)
                    yield tr_group("q", 2, 2, act_copy=True)
                    for c in range(1, 4):
                        yield prep_chunk(4 * c)
                        yield from tr_pairs(c)
                else:
                    # all preps first: they are cheap, their DMA deps landed
                    # long ago, and everything else queues behind them in
                    # each engine's in-order stream
                    for c in range(4):
                        yield prep_chunk(4 * c)
                    for c in range(4):
                        yield from tr_pairs(c)

            def job_chunks(h, mb):
                """Chunks of one (head, query-block) job, for interleaving."""
                kt, qt, vaug = heads[h]["kt"], heads[h]["qt"], heads[h]["vaug"]
                nsub = 4 * mb          # sub-diagonal key tiles
                rhs_q = qt[:, 4 * mb : 4 * mb + 4, :]    # [65, 512]
                jst = {"prev": None, "ot": None}

                def sub_group(s):
                    def run():
                        if jst["ot"] is None:
                            jst["ot"] = ot_pool.tile(
                                [P, 4, P], F32, tag="ot", name="ot"
                            )
                        stg = st_pool.tile([P, G, QB], F32, tag="stg")
                        for i in range(G):
                            nc.tensor.matmul(
                                stg[:, i, :], kt[:, s + i, :], rhs_q,
                                start=True, stop=True, skip_group_check=True,
                            )
                        pg = p_pool.tile([P, G, QB], HALF, tag="pg")
                        nc.scalar.activation(
                            pg[:], stg[:],
                            mybir.ActivationFunctionType.Exp, scale=0.25,
                        )
                        if jst["prev"] is not None:
                            _emit_pv(nc, jst["ot"], vaug, jst["prev"])
                        jst["prev"] = (pg, [s, s + 1])

                    return run

                def diag0_half(b):
                    # job (0,0) only: the first diag pair in two half-width
                    # pieces, so the very first exp waits on a 2-tile q DMA
                    # instead of the whole first quarter
                    def run():
                        if jst["ot"] is None:
                            jst["ot"] = ot_pool.tile(
                                [P, 4, P], F32, tag="ot", name="ot"
                            )
                        if b == 0:
                            jst["pgd"] = p_pool.tile(
                                [P, 4, QB], HALF, tag="pgd", name="pgd"
                            )
                            jst["stg0"] = st_pool.tile([P, G, QB], F32, tag="stg", name="stg")
                        pgd, stg = jst["pgd"], jst["stg0"]
                        cs = slice(2 * P * b, 2 * P * (b + 1))
                        for i in range(G):
                            # b=1 rides the pending-zero bytes of b=0's start
                            nc.tensor.matmul(
                                stg[:, i, cs], kt[:, i, :],
                                qt[:, 2 * b : 2 * b + 2, :],
                                start=(b == 0), stop=True,
                                skip_group_check=True,
                            )
                        nc.scalar.activation(
                            pgd[:, 0:2, cs], stg[:, :, cs],
                            mybir.ActivationFunctionType.Exp, scale=0.25,
                        )
                        if b == 0:
                            # both diag squares (tiles 0,1) lie in this half
                            for jj in (0, 1):
                                nc.vector.tensor_mul(
                                    out=pgd[:, jj, jj * P : (jj + 1) * P],
                                    in0=pgd[:, jj, jj * P : (jj + 1) * P],
                                    in1=tri[:],
                                )

                    return run

                def diag_group(a):
                    def run():
                        if jst["ot"] is None:
                            jst["ot"] = ot_pool.tile(
                                [P, 4, P], F32, tag="ot", name="ot"
                            )
                        if a == 0:
                            jst["pgd"] = p_pool.tile(
                                [P, 4, QB], HALF, tag="pgd", name="pgd"
                            )
                        pgd = jst["pgd"]
                        # columns m < 256a of tiles (2a, 2a+1) are fully
                        # masked: skip their QK matmul + exp; affine_select /
                        # dmask below zero-fill that (otherwise stale) region.
                        c0 = 2 * P * a
                        stg = st_pool.tile([P, G, QB], F32, tag="stg")
                        for i in range(G):
                            nc.tensor.matmul(
                                stg[:, i, c0:],
                                kt[:, 4 * mb + 2 * a + i, :],
                                qt[:, 4 * mb + 2 * a : 4 * mb + 4, :],
                                start=True, stop=True, skip_group_check=True,
                            )
                        nc.scalar.activation(
                            pgd[:, 2 * a : 2 * a + 2, c0:], stg[:, :, c0:],
                            mybir.ActivationFunctionType.Exp, scale=0.25,
                        )
                        # causal-mask each tile's diagonal 128x128 square (the
                        # only read region crossing the boundary)
                        for i in (0, 1):
                            jj = 2 * a + i
                            nc.vector.tensor_mul(
                                out=pgd[:, jj, jj * P : (jj + 1) * P],
                                in0=pgd[:, jj, jj * P : (jj + 1) * P],
                                in1=tri[:],
                            )

                    return run

                def pv_epilogue():
                    ot, pgd = jst["ot"], jst["pgd"]
                    if jst["prev"] is not None:
                        _emit_pv(nc, ot, vaug, jst["prev"])
                    # diagonal PV: query sub-tile j only takes contributions
                    # from diag tiles jj <= j (the rest are fully masked)
                    # PSUM zero-region semantics: start=True resets the whole
                    # 2KB bank, so only the very FIRST matmul into the ot bank
                    # may set it; later sub-tiles' first writes land on
                    # pending-zero bytes and overwrite (not accumulate).
                    # The very last job stores in two halves so its final DMA
                    # (a ~3.5us latency chain) starts earlier and is smaller.
                    last = False  # split-store off: cheap masks made its DVE insertions net-negative
                    for jj in range(4):
                        for j in range(jj, 4):
                            nc.tensor.matmul(
                                ot[:, j, :DE],
                                pgd[:, jj, j * P : (j + 1) * P],
                                vaug[:, 4 * mb + jj, :],
                                start=(nsub == 0 and jj == 0 and j == 0),
                                stop=(jj == j),
                                skip_group_check=True,
                            )
                        if last and jj == 1:
                            # j sub-tiles 0,1 are complete: flush them now
                            _emit_store(nc, out, epi_pool, ot, h, mb, 0, 2)
                    # epilogue: normalize + store (output is already in
                    # [query, d] orientation -- no transpose needed)
                    if last:
                        _emit_store(nc, out, epi_pool, ot, h, mb, 2, 4)
                    else:
                        _emit_store(nc, out, epi_pool, ot, h, mb, 0, 4)

                if h == 0 and mb == 0:
                    return [diag0_half(0), diag0_half(1), diag_group(1),
                            pv_epilogue]
                chunks = [sub_group(s) for s in range(0, nsub, G)]
                chunks += [diag_group(0), diag_group(1), pv_epilogue]
                return chunks

            # ---- software-pipelined emission: depth-2 job interleave ----
            # head 0: emit only the first two quarters' setup up front, drip
            # the rest between job chunks so the first QK isn't queued behind
            # every transpose on PE.  Emission order defines dependencies, so
            # job (0,mb) must have its quarters' setup emitted first:
            # h0 chunk list is [allocs, (prep,ktr,qtr) x 2 two-tile bites,
            # (prep, 4 tr pairs) x 3]; job (0,0) needs the first 4 chunks
            # (half-width diag), (0,mb>0) the first 7+5*mb.
            setup_q = {0: list(setup_chunks(0))}
            n0 = len(setup_q[0])
            for _ in range(7):
                setup_q[0].pop(0)()
            # later heads' setup chunks, dripped in ~1.5 heads ahead of use
            setup_q[1] = list(setup_chunks(1))

            def drip_one():
                for hh in sorted(setup_q):
                    if setup_q[hh]:
                        setup_q[hh].pop(0)()
                        return

            jobs = [(h, mb) for h in range(HPC) for mb in range(MBS)]
            active = []           # up to 2 jobs' chunk queues
            ji = 0
            while active or ji < len(jobs):
                while len(active) < 2 and ji < len(jobs):
                    h, mb = jobs[ji]
                    if h == 0:
                        while n0 - len(setup_q[0]) < (4 if mb == 0 else 7 + 5 * mb):
                            setup_q[0].pop(0)()
                    else:
                        # head h's setup must be fully emitted before its
                        # first job
                        if mb == 0:
                            for c in setup_q.get(h, []):
                                c()
                            setup_q[h] = []
                    if mb == 0 and h + 1 < HPC and h + 1 not in setup_q:
                        setup_q[h + 1] = list(setup_chunks(h + 1))
                    active.append(job_chunks(h, mb))
                    ji += 1
                for q_ in list(active):
                    q_.pop(0)()
                drip_one()
                drip_one()
                active = [q_ for q_ in active if q_]
            for hh in sorted(setup_q):
                for c in setup_q[hh]:
                    c()

    nc.compile()
    return nc


def _emit_store(nc, out, epi_pool, ot, h, mb, j0, j1):
    """Normalize query sub-tiles [j0, j1) of the accumulator and DMA them."""
    nj = j1 - j0
    linv = epi_pool.tile([P, 4], F32, tag="linv")
    nc.vector.reciprocal(linv[:, :nj], ot[:, j0:j1, D])
    o_sb = epi_pool.tile([P, 4, D], F32, tag="o_sb")
    nc.vector.tensor_mul(
        out=o_sb[:, :nj],
        in0=ot[:, j0:j1, :D],
        in1=linv[:, :nj, None].to_broadcast((P, nj, D)),
    )
    nc.sync.dma_start(
        out[h, mb * QB + j0 * P : mb * QB + j1 * P, :].rearrange(
            "(j p) d -> p j d", p=P
        ),
        o_sb[:, :nj],
    )


def _emit_pv(nc, ot, vaug, group):
    """PV for a full (unmasked) pair of key tiles: pg tiles are stationary
    [128 key, 128 query] operands, vaug [128 key, 65] moves."""
    pg, tiles = group
    for i, nt in enumerate(tiles):
        for j in range(4):
            # start only on the bank's very first matmul (see pv_epilogue)
            nc.tensor.matmul(
                ot[:, j, : D + 1],
                pg[:, i, j * P : (j + 1) * P],
                vaug[:, nt, :],
                start=(nt == 0 and j == 0),
                stop=False,
                skip_group_check=True,
            )


_NC = None


def _get_nc():
    global _NC
    if _NC is None:
        _NC = build_nc()
    return _NC


def kernel(q: np.ndarray, k: np.ndarray, v: np.ndarray) -> np.ndarray:
    from concourse.bass_utils import run_bass_kernel_spmd

    nc = _get_nc()
    qf = np.ascontiguousarray(np.asarray(q, dtype=np.float32).reshape(B * H, N, D))
    kf = np.ascontiguousarray(np.asarray(k, dtype=np.float32).reshape(B * H, N, D))
    vf = np.ascontiguousarray(np.asarray(v, dtype=np.float32).reshape(B * H, N, D))
    in_maps = [
        {
            "q": np.ascontiguousarray(qf[c * HPC : (c + 1) * HPC]),
            "k": np.ascontiguousarray(kf[c * HPC : (c + 1) * HPC]),
            "v": np.ascontiguousarray(vf[c * HPC : (c + 1) * HPC]),
        }
        for c in range(NCORES)
    ]
    res = run_bass_kernel_spmd(nc, in_maps, core_ids=list(range(NCORES)))
    outs = [res.results[c]["out"] for c in range(NCORES)]
    return np.concatenate(outs, axis=0).reshape(B, H, N, D)


if __name__ == "__main__":
    rng = np.random.default_rng(0)
    qq = rng.standard_normal((B, H, N, D), dtype=np.float32)
    kk = rng.standard_normal((B, H, N, D), dtype=np.float32)
    vv = rng.standard_normal((B, H, N, D), dtype=np.float32)
    o = kernel(q=qq, k=kk, v=vv)
    print("kernel ran, out shape", o.shape, "finite:", np.isfinite(o).all())
